# revision 1
# baseline (speedup 1.0000x reference)
"""PointNet-style encoder (conv1x1 stack + ragged segment-max) on 8 Trainium2 cores.

Strategy
--------
* BN folded into the conv weights host-side; every layer becomes matmul+bias+ReLU.
* Feature-major on device: activations live as [C, points] tiles, points stream
  through the PE as the matmul free dimension in 512-point macro-tiles.
* Raggedness handled host-side: each segment's points padded to a multiple of
  512 by duplicating its own points (exact under max-pooling), so every
  macro-tile belongs to exactly one segment. Segments are bin-packed 4 per
  core (sorted smallest-first, pad tiles assigned to the last slot so the
  tile order is slot-monotone), and every core is padded to the same
  macro-tile count Tm, so a single SPMD program covers all cores.
* The mid-network segment-max feeds back via the concat identity
  concat(f2, g) @ W3 = f2 @ W3[:256] + g @ W3[256:]; the g-contribution plus
  b3 becomes a per-macro-tile bias column (table U), applied by the activation
  instruction's per-partition bias operand.
* Phase A (layers 1-2, storing f2 in fp16 + per-tile maxes) and phase B
  (layers 3-4) are emitted as ONE interleaved pipeline: host-computed static
  bounds (slot s fully processed after phase-A tile bound[s] on every core)
  let per-slot g reductions and chunked U-table production run mid-stream, so
  phase A's DVE/ACT work hides entirely under phase B's PE-bound matmuls.
* Per-macro-tile layer-4 maxes are returned raw; the host applies
  relu(. + b4) and the per-segment max over tiles (monotonicity makes this
  exact).
* Matmuls run in float16 (fp32 PSUM accumulate): 1 cycle/column, ~1e-3 rel err.
  (float32r measured 2 cycles/column on HW: it is a 2-pass replicated mode.)
"""

import numpy as np

import concourse.bass as bass
import concourse.mybir as mybir
import concourse.tile as tile
from concourse import bacc
from concourse.bass_utils import run_bass_kernel_spmd

N_CORES = 8
PT = 512  # points per macro-tile
CH = 4  # U-table production chunk (tiles)
EPS = 1e-3  # keras BatchNormalization default epsilon

F32 = mybir.dt.float32
F16 = mybir.dt.float16
AF = mybir.ActivationFunctionType
AXX = mybir.AxisListType.X
ALU_MAX = mybir.AluOpType.max

_PROGRAM_CACHE: dict = {}


def _build_program(Tm: int, S: int, bounds: tuple, s_his: tuple):
    """One SPMD program for all cores: Tm macro-tiles, S segment slots.

    bounds[s]: phase-A tile count after which slot s is complete on every core.
    s_his[k]: max slot id appearing in tile chunk k (size CH) on any core.
    """
    nc = bacc.Bacc("TRN2")
    Tmp = Tm + (Tm % 2)
    nchunks = (Tm + CH - 1) // CH

    xT = nc.dram_tensor("xT", [3, Tm * PT], F16, kind="ExternalInput")
    mask = nc.dram_tensor("mask", [128, S, Tmp], F32, kind="ExternalInput")
    w1 = nc.dram_tensor("w1", [3, 128], F16, kind="ExternalInput")
    w2 = nc.dram_tensor("w2", [128, 2, 128], F16, kind="ExternalInput")
    w3a = nc.dram_tensor("w3a", [128, 2, 4, 128], F16, kind="ExternalInput")
    w3b = nc.dram_tensor("w3b", [128, 2, 4, 128], F16, kind="ExternalInput")
    w4 = nc.dram_tensor("w4", [128, 4, 8, 128], F16, kind="ExternalInput")
    b1 = nc.dram_tensor("b1", [128, 1], F32, kind="ExternalInput")
    b2 = nc.dram_tensor("b2", [128, 2], F32, kind="ExternalInput")
    b3 = nc.dram_tensor("b3", [128, 4], F32, kind="ExternalInput")
    mx4 = nc.dram_tensor("mx4", [128, 8, Tm], F32, kind="ExternalOutput")

    with tile.TileContext(nc) as tc:
        with (
            tc.tile_pool(name="const", bufs=1) as constp,
            tc.tile_pool(name="xp", bufs=4) as xp,
            tc.tile_pool(name="h1p", bufs=3) as h1p,
            tc.tile_pool(name="h3p", bufs=3) as h3p,
            tc.tile_pool(name="tmpp", bufs=4) as tmpp,
            tc.tile_pool(name="psA", bufs=2, space="PSUM") as psA,
            tc.tile_pool(name="psB3", bufs=2, space="PSUM") as psB3,
            tc.tile_pool(name="psB4", bufs=2, space="PSUM") as psB4,
        ):
            xTr0 = xT.ap()
            # prefetch the first x tiles ahead of the constant DMAs so the
            # first L1 matmul isn't queued behind them
            x_pre = {}
            for t0 in range(min(4, Tm)):
                x_sb0 = xp.tile([3, PT], F16, tag="x", name=f"x_{t0}")
                nc.sync.dma_start(out=x_sb0, in_=xTr0[:, t0 * PT : (t0 + 1) * PT])
                x_pre[t0] = x_sb0

            # small, immediately-needed constants on the sync DMA queue; the
            # big phase-B weights on the gpsimd queue so they don't
            # head-of-line-block phase A's x-tile loads.
            w1_sb = constp.tile([3, 128], F16)
            nc.sync.dma_start(out=w1_sb, in_=w1.ap())
            w2_sb = constp.tile([128, 2, 128], F16)
            nc.sync.dma_start(out=w2_sb, in_=w2.ap())
            b1_sb = constp.tile([128, 1], F32)
            nc.sync.dma_start(out=b1_sb, in_=b1.ap())
            b2_sb = constp.tile([128, 2], F32)
            nc.sync.dma_start(out=b2_sb, in_=b2.ap())
            b3_sb = constp.tile([128, 4], F32)
            nc.sync.dma_start(out=b3_sb, in_=b3.ap())
            mask_sb = constp.tile([128, S, Tmp], F32)
            nc.sync.dma_start(out=mask_sb, in_=mask.ap())
            w3a_sb = constp.tile([128, 2, 4, 128], F16)
            nc.gpsimd.dma_start(out=w3a_sb, in_=w3a.ap())
            w3b_sb = constp.tile([128, 2, 4, 128], F16)
            nc.gpsimd.dma_start(out=w3b_sb, in_=w3b.ap())
            w4_sb = constp.tile([128, 4, 8, 128], F16)
            nc.gpsimd.dma_start(out=w4_sb, in_=w4.ap())

            f2_all = constp.tile([128, Tm, 2, PT], F16)
            Mx2_sb = constp.tile([128, 2, Tm], F32)
            g_sb = constp.tile([128, 2, S], F32)
            Gacc_sb = constp.tile([128, 2, Tmp], F32)
            G2_sb = constp.tile([128, 2, Tmp], F16)
            U_sb = constp.tile([128, 4, Tmp], F32)
            Mx4_sb = constp.tile([128, 8, Tm], F32)

            xTr = xT.ap()

            # HAM warmup: dependency-free matmuls keep the PE busy through the
            # DMA prologue so the clock gate opens (1.2 -> 2.4 GHz) before the
            # real matmuls start.
            warm_src = constp.tile([128, PT], F16, name="warm_src")
            nc.vector.memset(warm_src, 0.01)
            warm_out = constp.tile([128, 1], F32, name="warm_out")
            # touch the scalar engine early so its ACT_TABLE_LOAD happens
            # during the DMA prologue instead of gating the first real Relu
            warm_act = constp.tile([128, 8], F32, name="warm_act")
            nc.scalar.activation(out=warm_act, in_=warm_src[:, 0:8], func=AF.Relu)
            ps_w = psA.tile([128, PT], F32, tag="psa", name="ps_warm")
            for i in range(20):
                nc.tensor.matmul(
                    ps_w[:, :], warm_src[:, 0:128], warm_src[:, :], start=True, stop=True
                )
            nc.vector.tensor_reduce(out=warm_out, in_=ps_w[:, 0:8], axis=AXX, op=ALU_MAX)

            # ---------------- emission helpers ----------------
            deferred_reduce: list = []

            def emit_A(t, defer_reduce=False):
                """L1+L2 for tile t; stores f2 (fp16) and its per-tile max."""
                if t in x_pre:
                    x_sb = x_pre.pop(t)
                else:
                    x_sb = xp.tile([3, PT], F16, tag="x", name=f"x_{t}")
                    nc.sync.dma_start(out=x_sb, in_=xTr[:, t * PT : (t + 1) * PT])
                ps1 = psA.tile([128, PT], F32, tag="psa", name=f"ps1_{t}")
                nc.tensor.matmul(ps1[:, :], w1_sb[:, :], x_sb[:, :], start=True, stop=True)
                h1_sb = h1p.tile([128, PT], F16, tag="h1", name=f"h1_{t}")
                nc.scalar.activation(out=h1_sb, in_=ps1, func=AF.Relu, bias=b1_sb[:, 0:1])
                for c in range(2):
                    ps2 = psA.tile([128, PT], F32, tag="psa", name=f"ps2_{t}_{c}")
                    nc.tensor.matmul(ps2[:, :], w2_sb[:, c, :], h1_sb[:, :], start=True, stop=True)
                    if c == 0:
                        nc.scalar.activation(
                            out=f2_all[:, t, c, :], in_=ps2, func=AF.Relu, bias=b2_sb[:, c : c + 1]
                        )
                    else:
                        # relu(x + b) on the DVE to balance ACT/DVE load
                        nc.vector.tensor_scalar(
                            f2_all[:, t, c, :], ps2, b2_sb[:, c : c + 1], 0.0,
                            mybir.AluOpType.add, ALU_MAX,
                        )
                if defer_reduce:
                    deferred_reduce.append(t)
                else:
                    nc.vector.tensor_reduce(
                        out=Mx2_sb[:, :, t : t + 1], in_=f2_all[:, t, :, :], axis=AXX, op=ALU_MAX
                    )

            def emit_g(s):
                """Per-slot max over the (host-bounded) range of Mx2 columns."""
                b = bounds[s]
                for c in range(2):
                    tmp = tmpp.tile([128, Tm], F32, tag="tmp", name=f"tmpg_{c}_{s}")
                    nc.vector.tensor_mul(tmp[:, :b], Mx2_sb[:, c, :b], mask_sb[:, s, :b])
                    nc.vector.tensor_reduce(
                        out=g_sb[:, c, s : s + 1], in_=tmp[:, :b], axis=AXX, op=ALU_MAX
                    )

            def emit_Uchunk(k):
                """U[:, :, kCH:kCH+w] = W3b.T @ G2_chunk + b3 (per-tile bias)."""
                c0 = k * CH
                w = min(CH, Tm - c0)
                we = w + (w % 2)  # keep matmul free dims even
                shi = s_his[k]
                for c in range(2):
                    nc.vector.tensor_scalar_mul(
                        Gacc_sb[:, c, c0 : c0 + we], mask_sb[:, 0, c0 : c0 + we], g_sb[:, c, 0:1]
                    )
                    for s in range(1, shi + 1):
                        tmp2 = tmpp.tile([128, CH + 1], F32, tag="tmp2", name=f"tmpe_{k}_{c}_{s}")
                        nc.vector.tensor_scalar_mul(
                            tmp2[:, :we], mask_sb[:, s, c0 : c0 + we], g_sb[:, c, s : s + 1]
                        )
                        nc.vector.tensor_add(
                            Gacc_sb[:, c, c0 : c0 + we], Gacc_sb[:, c, c0 : c0 + we], tmp2[:, :we]
                        )
                    nc.scalar.copy(G2_sb[:, c, c0 : c0 + we], Gacc_sb[:, c, c0 : c0 + we])
                for m in range(4):
                    psu = psA.tile([128, PT], F32, tag="psa", name=f"psu_{k}_{m}")
                    nc.tensor.matmul(
                        psu[:, :we], w3b_sb[:, 0, m, :], G2_sb[:, 0, c0 : c0 + we],
                        start=True, stop=False,
                    )
                    nc.tensor.matmul(
                        psu[:, :we], w3b_sb[:, 1, m, :], G2_sb[:, 1, c0 : c0 + we],
                        start=False, stop=True,
                    )
                    nc.scalar.activation(
                        out=U_sb[:, m, c0 : c0 + we], in_=psu[:, :we],
                        func=AF.Identity, bias=b3_sb[:, m : m + 1],
                    )

            h3_tiles = {}

            def emit_L3(t):
                h3_sb = h3p.tile([128, 4, PT], F16, tag="h3", name=f"h3_{t}")
                for m in range(4):
                    ps3 = psB3.tile([128, PT], F32, tag="ps3", name=f"ps3_{t}_{m}")
                    nc.tensor.matmul(
                        ps3[:, :], w3a_sb[:, 0, m, :], f2_all[:, t, 0, :], start=True, stop=False
                    )
                    nc.tensor.matmul(
                        ps3[:, :], w3a_sb[:, 1, m, :], f2_all[:, t, 1, :], start=False, stop=True
                    )
                    nc.scalar.activation(
                        out=h3_sb[:, m, :], in_=ps3, func=AF.Relu, bias=U_sb[:, m, t : t + 1]
                    )
                h3_tiles[t] = h3_sb

            def emit_L4(t):
                h3_sb = h3_tiles.pop(t)
                for mg in range(4):
                    # inner dim padded to a full PSUM bank (512 f32) so each
                    # m-chunk's matmul output stays within one bank
                    ps4 = psB4.tile([128, 2, 512], F32, tag="ps4", name=f"ps4_{t}_{mg}")
                    for mi in range(2):
                        m = mg * 2 + mi
                        for k in range(4):
                            nc.tensor.matmul(
                                ps4[:, mi, :PT], w4_sb[:, k, m, :], h3_sb[:, k, :],
                                start=(k == 0), stop=(k == 3),
                            )
                    nc.vector.tensor_reduce(
                        out=Mx4_sb[:, 2 * mg : 2 * mg + 2, t : t + 1], in_=ps4[:, :, :PT], axis=AXX, op=ALU_MAX
                    )

            # ---------------- interleaved pipeline ----------------
            a_next = 0
            b_next = 0
            l3_next = 0
            u_next = 0
            g_emitted = [False] * S

            def try_unlock():
                nonlocal u_next
                for s in range(S):
                    if not g_emitted[s] and a_next >= bounds[s]:
                        # flush reduces this slot's g depends on
                        for t in [d for d in deferred_reduce if d < bounds[s]]:
                            nc.vector.tensor_reduce(
                                out=Mx2_sb[:, :, t : t + 1], in_=f2_all[:, t, :, :],
                                axis=AXX, op=ALU_MAX,
                            )
                            deferred_reduce.remove(t)
                        emit_g(s)
                        g_emitted[s] = True
                while u_next < nchunks and all(g_emitted[s] for s in range(s_his[u_next] + 1)):
                    emit_Uchunk(u_next)
                    u_next += 1

            # phase A must lead phase B by enough tiles that B's U-table
            # chunks are always unlocked when its L3s reach the PE queue
            need = [bounds[s_his[min(i + 1, Tm - 1) // CH]] for i in range(Tm)]
            LEAD = max(max(need[i] - i for i in range(Tm)) + 1, need[0])

            # the fill phase is DVE/ACT-paced with the PE at ~50% duty, which
            # re-throttles the clock gate; pad it with dummy matmuls into a
            # psB4-pool tile (idle until the first L4, released before the
            # second one needs its slot)
            ps_dummy = psB4.tile([128, 2, 512], F32, tag="ps4", name="ps_dummy")

            while b_next < Tm:
                while a_next < min(Tm, b_next + LEAD):
                    # fill-tail tiles (beyond slot 0 on every core) defer their
                    # DVE reduce into the stream's slack
                    emit_A(a_next, defer_reduce=(b_next == 0 and a_next >= bounds[0]))
                    if b_next == 0:
                        for _ in range(4):
                            nc.tensor.matmul(
                                ps_dummy[:, 0, :PT], warm_src[:, 0:128], warm_src[:, :],
                                start=True, stop=True,
                            )
                    a_next += 1
                    try_unlock()
                progressed = False
                while (
                    l3_next <= min(b_next + 1, Tm - 1)
                    and l3_next // CH < u_next
                    and l3_next < a_next
                ):
                    emit_L3(l3_next)
                    l3_next += 1
                    progressed = True
                if l3_next > b_next:
                    if b_next == 0:
                        # bridge the prologue stall (L4(0) waiting on the first
                        # h3 activations) so the clock gate stays open
                        for _ in range(14):
                            nc.tensor.matmul(
                                ps_dummy[:, 0, :PT], warm_src[:, 0:128], warm_src[:, :],
                                start=True, stop=True,
                            )
                    emit_L4(b_next)
                    b_next += 1
                    progressed = True
                if not progressed:
                    if a_next < Tm:
                        emit_A(a_next)
                        a_next += 1
                        try_unlock()
                    else:
                        raise RuntimeError("pipeline deadlock")

            nc.sync.dma_start(out=mx4.ap(), in_=Mx4_sb)

    nc.finalize()
    return nc


def _prepare(x, seg_ids, B):
    """Pad + pack segments into per-core, slot-monotone macro-tile streams."""
    counts = np.bincount(seg_ids, minlength=B)
    starts = np.concatenate([[0], np.cumsum(counts)])
    seg_tiles = [(int(c) + PT - 1) // PT for c in counts]

    SLOTS = (B + N_CORES - 1) // N_CORES
    order = np.argsort(-np.asarray(seg_tiles), kind="stable")
    core_segs: list[list[int]] = [[] for _ in range(N_CORES)]
    core_load = [0] * N_CORES
    for s in order:
        cands = [c for c in range(N_CORES) if len(core_segs[c]) < SLOTS]
        c = min(cands, key=lambda i: core_load[i])
        core_segs[c].append(int(s))
        core_load[c] += seg_tiles[s]

    # local search: swap segments between cores to shave the max load
    ideal = (sum(seg_tiles) + N_CORES - 1) // N_CORES
    for _ in range(200):
        if max(core_load) <= ideal:
            break
        hi = max(range(N_CORES), key=lambda i: core_load[i])
        improved = False
        for lo in sorted(range(N_CORES), key=lambda i: core_load[i]):
            if lo == hi:
                continue
            for ia, sa in enumerate(core_segs[hi]):
                for ib, sb in enumerate(core_segs[lo]):
                    d = seg_tiles[sa] - seg_tiles[sb]
                    if d > 0 and max(core_load[hi] - d, core_load[lo] + d) < max(
                        core_load[hi], core_load[lo]
                    ):
                        core_segs[hi][ia], core_segs[lo][ib] = sb, sa
                        core_load[hi] -= d
                        core_load[lo] += d
                        improved = True
                        break
                if improved:
                    break
            if improved:
                break
        if not improved:
            break
    Tm = max(core_load)

    # order each core's slots to directly minimize the pipeline LEAD
    # (phase-A tiles that must precede phase B); coordinate descent over
    # cores, starting from a cumsum-balanced ordering
    from itertools import permutations

    nchunks = (Tm + CH - 1) // CH

    def _sot_for(perm, load):
        sot = []
        for slot, s in enumerate(perm):
            sot += [slot] * seg_tiles[s]
        sot += [SLOTS - 1] * (Tm - load)
        return np.asarray(sot)

    def _lead_for(sots_list):
        bb = [
            int(max(np.flatnonzero(st == s).max() for st in sots_list)) + 1
            for s in range(SLOTS)
        ]
        sh = [
            int(max(st[k * CH : min((k + 1) * CH, Tm)].max() for st in sots_list))
            for k in range(nchunks)
        ]
        nd = [bb[sh[min(i + 1, Tm - 1) // CH]] for i in range(Tm)]
        return max(max(nd[i] - i for i in range(Tm)) + 1, nd[0]), sum(bb)

    targets = [Tm * (i + 1) / SLOTS for i in range(SLOTS)]
    for c in range(N_CORES):
        best, best_score = None, None
        for perm in permutations(core_segs[c]):
            cs, score = 0, 0.0
            for i, s in enumerate(perm):
                cs += seg_tiles[s]
                score += abs(cs - targets[i])
            if best_score is None or score < best_score:
                best, best_score = perm, score
        core_segs[c] = list(best)

    cur_sots = [_sot_for(core_segs[c], core_load[c]) for c in range(N_CORES)]
    for _sweep in range(3):
        changed = False
        for c in range(N_CORES):
            best_perm, best_key = tuple(core_segs[c]), _lead_for(cur_sots)
            for perm in permutations(core_segs[c]):
                cur_sots[c] = _sot_for(perm, core_load[c])
                key = _lead_for(cur_sots)
                if key < best_key:
                    best_key, best_perm = key, perm
            core_segs[c] = list(best_perm)
            cur_sots[c] = _sot_for(best_perm, core_load[c])
            changed = changed or tuple(core_segs[c]) != best_perm
        if not changed:
            break

    xT_cores, mask_cores, post = [], [], []
    sots = []
    for c in range(N_CORES):
        pts_list, slot_of_tile = [], []
        for slot, s in enumerate(core_segs[c]):
            seg_pts = x[starts[s] : starts[s + 1]]
            ntile = seg_tiles[s]
            padn = ntile * PT - len(seg_pts)
            if padn:
                seg_pts = np.concatenate([seg_pts, seg_pts[:padn]])
            pts_list.append(seg_pts)
            slot_of_tile += [slot] * ntile
        extra = Tm - core_load[c]
        if extra:
            # core-equalization pad tiles duplicate the LAST slot's points so
            # the tile order stays slot-monotone
            pts_list.append(np.tile(pts_list[-1][:PT], (extra, 1)))
            slot_of_tile += [SLOTS - 1] * extra
        xc = np.concatenate(pts_list).astype(np.float16)
        xT_cores.append(np.ascontiguousarray(xc.T))
        sot = np.asarray(slot_of_tile)
        sots.append(sot)
        Tmp = Tm + (Tm % 2)
        m01 = np.zeros((SLOTS, Tmp), np.float32)
        m01[:, :Tm] = sot[None, :] == np.arange(SLOTS)[:, None]
        mask_cores.append(np.ascontiguousarray(np.broadcast_to(m01[None], (128, SLOTS, Tmp))))
        post.append((core_segs[c], sot))

    # static pipeline bounds (shared across cores)
    bounds = tuple(
        int(max(np.flatnonzero(sot == s).max() for sot in sots)) + 1 for s in range(SLOTS)
    )
    nchunks = (Tm + CH - 1) // CH
    s_his = tuple(
        int(max(sot[k * CH : min((k + 1) * CH, Tm)].max() for sot in sots))
        for k in range(nchunks)
    )
    return Tm, SLOTS, xT_cores, mask_cores, post, bounds, s_his


def make_in_maps(inputs):
    """Fold BN, pack points, and build the per-core SPMD input dicts.

    Returns (key, in_maps, post, b4f) where key indexes _PROGRAM_CACHE.
    """
    x = np.asarray(inputs["x"], np.float32)
    seg_ids = np.asarray(inputs["seg_ids"])
    B = int(inputs["num_segments"])

    Wf, bf = [], []
    for i in (1, 2, 3, 4):
        W = np.asarray(inputs[f"W{i}"], np.float32)
        b = np.asarray(inputs[f"b{i}"], np.float32)
        ga = np.asarray(inputs[f"g{i}"], np.float32)
        be = np.asarray(inputs[f"be{i}"], np.float32)
        m = np.asarray(inputs[f"m{i}"], np.float32)
        v = np.asarray(inputs[f"v{i}"], np.float32)
        sc = ga / np.sqrt(v + EPS)
        Wf.append(np.ascontiguousarray(W * sc[None, :]))
        bf.append((b - m) * sc + be)
    W1f, W2f, W3f, W4f = Wf
    b1f, b2f, b3f, b4f = bf

    Tm, SLOTS, xT_cores, mask_cores, post, bounds, s_his = _prepare(x, seg_ids, B)

    w1d = W1f.astype(np.float16)
    w2d = np.ascontiguousarray(W2f.reshape(128, 2, 128).astype(np.float16))
    w3ad = np.ascontiguousarray(W3f[:256].reshape(2, 128, 4, 128).transpose(1, 0, 2, 3).astype(np.float16))
    w3bd = np.ascontiguousarray(W3f[256:].reshape(2, 128, 4, 128).transpose(1, 0, 2, 3).astype(np.float16))
    w4d = np.ascontiguousarray(W4f.reshape(4, 128, 8, 128).transpose(1, 0, 2, 3).astype(np.float16))
    b1d = np.ascontiguousarray(b1f.reshape(128, 1))
    b2d = np.ascontiguousarray(b2f.reshape(2, 128).T)
    b3d = np.ascontiguousarray(b3f.reshape(4, 128).T)

    in_maps = [
        {
            "xT": xT_cores[c],
            "mask": mask_cores[c],
            "w1": w1d,
            "w2": w2d,
            "w3a": w3ad,
            "w3b": w3bd,
            "w4": w4d,
            "b1": b1d,
            "b2": b2d,
            "b3": b3d,
        }
        for c in range(N_CORES)
    ]
    return (Tm, SLOTS, bounds, s_his), in_maps, post, b4f


def postprocess(results, post, b4f, B):
    out = np.zeros((B, 1024), np.float32)
    for c in range(N_CORES):
        mx4 = results[c]["mx4"]  # [128, 8, Tm]
        segs, sot = post[c]
        for slot, s in enumerate(segs):
            cols = np.flatnonzero(sot == slot)
            raw = mx4[:, :, cols].max(axis=2)  # [128, 8]
            out[s] = np.maximum(raw.T.reshape(1024) + b4f, 0.0)
    return out


def get_program(key):
    if key not in _PROGRAM_CACHE:
        _PROGRAM_CACHE[key] = _build_program(*key)
    return _PROGRAM_CACHE[key]


def kernel(**inputs) -> np.ndarray:
    B = int(inputs["num_segments"])
    key, in_maps, post, b4f = make_in_maps(inputs)
    nc = get_program(key)
    last_err = None
    for _ in range(3):  # retry transient NRT device wedges
        try:
            res = run_bass_kernel_spmd(nc, in_maps, core_ids=list(range(N_CORES)))
            return postprocess(res.results, post, b4f, B)
        except Exception as e:  # noqa: BLE001
            last_err = e
    raise last_err



# revision 3
# speedup vs baseline: 1.0344x; 1.0344x over previous
"""PointNet-style encoder (conv1x1 stack + ragged segment-max) on 8 Trainium2 cores.

Strategy (v2 — tail-tile packing)
---------------------------------
* BN folded into conv weights host-side; every layer is matmul+bias+ReLU.
* Feature-major on device: activations live as [C, points] tiles; points stream
  through the PE as the matmul free dimension in 512-point macro-tiles.
* Segments are point-balanced across the 8 cores (whole segments per core, so
  the two segment-maxes stay core-local). Per core the layout is:
    - T_TAIL fixed "tail" tiles (array idx 0..T_TAIL-1) holding every
      segment's sub-512 remainder, packed as 64-col-aligned per-slot chunks;
    - P_PURE "pure" tiles (idx T_TAIL..) each belonging to one segment.
  This wastes ~1 tile/core instead of the ~3 of per-segment padding.
* Phase A (L1+L2) runs tail tiles first, then pures in slot order; per-tile
  f2 maxes (Mx2) land in a combined column buffer: 8 per-64-col group maxes
  per tail tile, 1 per pure tile. Per-slot g = masked max over that buffer
  (gmask input zeros out other slots; buffer memset to 0 and f2>=0 keep the
  not-yet-written columns neutral).
* Mid-network unpool via concat identity: concat(f2, g)@W3 = f2@W3a + g@W3b.
  Per slot Us = W3b^T g + b3 (tiny 1-col matmuls); pure tiles get a per-tile
  bias column Ub[:,m,j] = Us[:,m,slot(j)] built by 2 tiny DVE ops (umaskT
  input). Tail tiles get the g-term exactly via an extra accumulated matmul
  per m-chunk: lhsT = UsT (= (W3b^T G2)^T, computed as G2^T@W3b, [S,512]),
  rhs = one-hot slot matrix O [S, cols] — per-point unpool without masks.
* Phase B (L3+L4) runs pures first (unlocked per slot as g arrives), tail
  tiles last (they need all slots' UsT). L4 maxes are reduced per pure tile
  and per 64-col group for tail tiles (raw, pre-bias); the host applies
  relu(.+b4) and combines columns per segment (exact under max).
* Single interleaved pipeline (phase A runs LEAD tiles ahead of phase B) so
  A's ACT/DVE drains hide under B's PE-bound matmuls.
* Matmuls in float16 (fp32 PSUM): 1 cycle/column, ~1e-3 rel err.
"""

import numpy as np

import concourse.bass as bass
import concourse.mybir as mybir
import concourse.tile as tile
from concourse import bacc
from concourse.bass_utils import run_bass_kernel_spmd

N_CORES = 8
PT = 512
GRP = 64  # tail group granularity (cols)
NG = PT // GRP  # groups per tile
EPS = 1e-3  # keras BatchNormalization default epsilon

F32 = mybir.dt.float32
F16 = mybir.dt.float16
AF = mybir.ActivationFunctionType
AXX = mybir.AxisListType.X
AXXY = mybir.AxisListType.XY
ALU_MAX = mybir.AluOpType.max
ALU_ADD = mybir.AluOpType.add

_PROGRAM_CACHE: dict = {}


def _build_program(T_tail: int, P_pure: int, S: int, bounds: tuple, shi_pure: tuple):
    """One SPMD program for all cores.

    bounds[s]: #A-tiles after which slot s's f2 is complete on every core.
    shi_pure[j]: max (over cores) slot id of pure tile j.
    """
    TM = T_tail + P_pure
    NTC = T_tail * NG  # tail group columns
    GW = NTC + P_pure  # combined max-buffer width

    nc = bacc.Bacc("TRN2")

    xT = nc.dram_tensor("xT", [3, TM * PT], F16, kind="ExternalInput")
    gmask = nc.dram_tensor("gmask", [128, S, GW], F32, kind="ExternalInput")
    umaskT = nc.dram_tensor("umaskT", [128, P_pure, S], F32, kind="ExternalInput")
    onehot = nc.dram_tensor("onehot", [S, T_tail * PT], F16, kind="ExternalInput")
    w1 = nc.dram_tensor("w1", [3, 128], F16, kind="ExternalInput")
    w2 = nc.dram_tensor("w2", [128, 2, 128], F16, kind="ExternalInput")
    w3a = nc.dram_tensor("w3a", [128, 2, 4, 128], F16, kind="ExternalInput")
    w3b = nc.dram_tensor("w3b", [128, 2, 4, 128], F16, kind="ExternalInput")
    w4 = nc.dram_tensor("w4", [128, 4, 8, 128], F16, kind="ExternalInput")
    b1 = nc.dram_tensor("b1", [128, 1], F32, kind="ExternalInput")
    b2 = nc.dram_tensor("b2", [128, 2], F32, kind="ExternalInput")
    b3 = nc.dram_tensor("b3", [128, 4], F32, kind="ExternalInput")
    mx4 = nc.dram_tensor("mx4", [128, 8, GW], F32, kind="ExternalOutput")

    with tile.TileContext(nc) as tc:
        with (
            tc.tile_pool(name="const", bufs=1) as constp,
            tc.tile_pool(name="h1p", bufs=3) as h1p,
            tc.tile_pool(name="h3p", bufs=3) as h3p,
            tc.tile_pool(name="tmpp", bufs=4) as tmpp,
            tc.tile_pool(name="psA", bufs=3, space="PSUM") as psA,
            tc.tile_pool(name="psB3", bufs=2, space="PSUM") as psB3,
            tc.tile_pool(name="psB4", bufs=2, space="PSUM") as psB4,
            tc.tile_pool(name="psUT", bufs=1, space="PSUM") as psUT,
        ):
            # x first on the sync queue so the first L1 matmul unblocks ASAP
            x_sb = constp.tile([3, TM * PT], F16)
            nc.sync.dma_start(out=x_sb, in_=xT.ap())
            w1_sb = constp.tile([3, 128], F16)
            nc.sync.dma_start(out=w1_sb, in_=w1.ap())
            w2_sb = constp.tile([128, 2, 128], F16)
            nc.sync.dma_start(out=w2_sb, in_=w2.ap())
            b1_sb = constp.tile([128, 1], F32)
            nc.sync.dma_start(out=b1_sb, in_=b1.ap())
            b2_sb = constp.tile([128, 2], F32)
            nc.sync.dma_start(out=b2_sb, in_=b2.ap())
            b3_sb = constp.tile([128, 4], F32)
            nc.sync.dma_start(out=b3_sb, in_=b3.ap())
            gmask_sb = constp.tile([128, S, GW], F32)
            nc.sync.dma_start(out=gmask_sb, in_=gmask.ap())
            umaskT_sb = constp.tile([128, P_pure, S], F32)
            nc.sync.dma_start(out=umaskT_sb, in_=umaskT.ap())
            onehot_sb = constp.tile([S, T_tail * PT], F16)
            nc.sync.dma_start(out=onehot_sb, in_=onehot.ap())
            # big phase-B weights on the gpsimd queue so they don't block x
            w3a_sb = constp.tile([128, 2, 4, 128], F16)
            nc.gpsimd.dma_start(out=w3a_sb, in_=w3a.ap())
            w3b_sb = constp.tile([128, 2, 4, 128], F16)
            nc.gpsimd.dma_start(out=w3b_sb, in_=w3b.ap())
            w4_sb = constp.tile([128, 4, 8, 128], F16)
            nc.gpsimd.dma_start(out=w4_sb, in_=w4.ap())

            f2_all = constp.tile([128, TM, 2, NG, GRP], F16)
            Mx2_sb = constp.tile([128, 2, GW], F32)
            g_sb = constp.tile([128, 2, S], F32)
            G2s_sb = constp.tile([128, 2, S], F16)
            Us_sb = constp.tile([128, 4, S], F32)
            Ub_sb = constp.tile([128, 4, P_pure], F32)
            UsT_sb = constp.tile([S, 4, 128], F16)
            Mx4_sb = constp.tile([128, 8, GW], F32)

            # zero-init buffers that masked ops may read before they're fully
            # written (0 is neutral: f2 >= 0 and gmask/umaskT are 0 there)
            nc.vector.memset(Mx2_sb, 0.0)
            nc.vector.memset(Us_sb, 0.0)
            nc.vector.memset(G2s_sb, 0.0)

            # HAM warmup: dependency-free matmuls during the DMA prologue so
            # the PE clock gate opens before the real stream starts
            warm_src = constp.tile([128, PT], F16, name="warm_src")
            nc.vector.memset(warm_src, 0.01)
            warm_out = constp.tile([128, 1], F32, name="warm_out")
            warm_act = constp.tile([128, 8], F32, name="warm_act")
            nc.scalar.activation(out=warm_act, in_=warm_src[:, 0:8], func=AF.Relu)
            ps_w = psB4.tile([128, NG, GRP], F32, tag="ps4", name="ps_warm")
            for _ in range(10):
                nc.tensor.matmul(
                    ps_w[:, :, :], warm_src[:, 0:128], warm_src[:, :], start=True, stop=True
                )
            nc.vector.tensor_reduce(out=warm_out, in_=ps_w[:, 0, 0:8], axis=AXX, op=ALU_MAX)

            xr = x_sb

            # ---------------- emission helpers ----------------
            def emit_A(t):
                """L1+L2 for array tile t; stores f2 (fp16) + its Mx2 column(s)."""
                ps1 = psA.tile([128, NG, GRP], F32, tag="psa", name=f"ps1_{t}")
                nc.tensor.matmul(
                    ps1[:, :, :], w1_sb[:, :], xr[:, t * PT : (t + 1) * PT],
                    start=True, stop=True,
                )
                h1_sb = h1p.tile([128, NG, GRP], F16, tag="h1", name=f"h1_{t}")
                nc.scalar.activation(out=h1_sb, in_=ps1, func=AF.Relu, bias=b1_sb[:, 0:1])
                for c in range(2):
                    ps2 = psA.tile([128, NG, GRP], F32, tag="psa", name=f"ps2_{t}_{c}")
                    nc.tensor.matmul(
                        ps2[:, :, :], w2_sb[:, c, :], h1_sb[:, :, :], start=True, stop=True
                    )
                    if c == 0:
                        nc.scalar.activation(
                            out=f2_all[:, t, c], in_=ps2, func=AF.Relu,
                            bias=b2_sb[:, c : c + 1],
                        )
                    else:
                        nc.vector.tensor_scalar(
                            f2_all[:, t, c], ps2, b2_sb[:, c : c + 1], 0.0,
                            ALU_ADD, ALU_MAX,
                        )
                if t < T_tail:
                    # per-64-col group maxes
                    nc.vector.tensor_reduce(
                        out=Mx2_sb[:, :, t * NG : (t + 1) * NG], in_=f2_all[:, t],
                        axis=AXX, op=ALU_MAX,
                    )
                else:
                    j = t - T_tail
                    nc.vector.tensor_reduce(
                        out=Mx2_sb[:, :, NTC + j : NTC + j + 1], in_=f2_all[:, t],
                        axis=AXXY, op=ALU_MAX,
                    )

            def emit_g(s):
                """g[s] = masked max over the Mx2 column buffer, then Us[s]."""
                for c in range(2):
                    tmp = tmpp.tile([128, GW], F32, tag="tmp", name=f"tmpg_{c}_{s}")
                    nc.vector.tensor_mul(tmp[:, :], Mx2_sb[:, c, :], gmask_sb[:, s, :])
                    nc.vector.tensor_reduce(
                        out=g_sb[:, c, s : s + 1], in_=tmp[:, :], axis=AXX, op=ALU_MAX
                    )
                nc.scalar.copy(G2s_sb[:, :, s], g_sb[:, :, s])
                psu = psA.tile([128, NG, GRP], F32, tag="psa", name=f"psu_{s}")
                for m in range(4):
                    nc.tensor.matmul(
                        psu[:, m, s : s + 1], w3b_sb[:, 0, m, :], G2s_sb[:, 0, s : s + 1],
                        start=True, stop=False,
                    )
                    nc.tensor.matmul(
                        psu[:, m, s : s + 1], w3b_sb[:, 1, m, :], G2s_sb[:, 1, s : s + 1],
                        start=False, stop=True,
                    )
                for m in range(4):
                    nc.scalar.activation(
                        out=Us_sb[:, m, s : s + 1], in_=psu[:, m, s : s + 1],
                        func=AF.Identity, bias=b3_sb[:, m : m + 1],
                    )

            def emit_UsT():
                """UsT = (G2s)^T @ W3b  -> [S, 512] fp16 (for tail unpool)."""
                ps = psUT.tile([128, 4, 128], F32, tag="psut", name="ps_ust")
                nc.tensor.matmul(
                    ps[0:S, :, :], G2s_sb[:, 0, 0:S], w3b_sb[:, 0], start=True, stop=False
                )
                nc.tensor.matmul(
                    ps[0:S, :, :], G2s_sb[:, 1, 0:S], w3b_sb[:, 1], start=False, stop=True
                )
                nc.scalar.copy(UsT_sb[0:S], ps[0:S, :, :])

            def emit_Ub(j):
                """Per-tile L3 bias column Ub[:,m,j] = Us[:,m,slot(j)] (+b3)."""
                for m in range(4):
                    tmp = tmpp.tile([128, S], F32, tag="tmpu", name=f"tmpu_{j}_{m}")
                    nc.vector.tensor_mul(tmp[:, :], Us_sb[:, m, :], umaskT_sb[:, j, :])
                    nc.vector.tensor_reduce(
                        out=Ub_sb[:, m, j : j + 1], in_=tmp[:, :], axis=AXX, op=ALU_ADD
                    )

            h3_tiles = {}

            def emit_L3(t):
                h3_sb = h3p.tile([128, 4, NG, GRP], F16, tag="h3", name=f"h3_{t}")
                tail = t < T_tail
                for m in range(4):
                    ps3 = psB3.tile([128, NG, GRP], F32, tag="ps3", name=f"ps3_{t}_{m}")
                    nc.tensor.matmul(
                        ps3[:, :, :], w3a_sb[:, 0, m, :], f2_all[:, t, 0],
                        start=True, stop=False,
                    )
                    nc.tensor.matmul(
                        ps3[:, :, :], w3a_sb[:, 1, m, :], f2_all[:, t, 1],
                        start=False, stop=not tail,
                    )
                    if tail:
                        nc.tensor.matmul(
                            ps3[:, :, :], UsT_sb[0:S, m, :],
                            onehot_sb[0:S, t * PT : (t + 1) * PT],
                            start=False, stop=True,
                        )
                        bias = b3_sb[:, m : m + 1]
                    else:
                        bias = Ub_sb[:, m, t - T_tail : t - T_tail + 1]
                    nc.scalar.activation(
                        out=h3_sb[:, m], in_=ps3, func=AF.Relu, bias=bias
                    )
                h3_tiles[t] = h3_sb

            def emit_L4(t):
                h3_sb = h3_tiles.pop(t)
                for mi in range(8):
                    ps4 = psB4.tile([128, NG, GRP], F32, tag="ps4", name=f"ps4_{t}_{mi}")
                    for k in range(4):
                        nc.tensor.matmul(
                            ps4[:, :, :], w4_sb[:, k, mi, :], h3_sb[:, k],
                            start=(k == 0), stop=(k == 3),
                        )
                    if t < T_tail:
                        nc.vector.tensor_reduce(
                            out=Mx4_sb[:, mi, t * NG : (t + 1) * NG], in_=ps4,
                            axis=AXX, op=ALU_MAX,
                        )
                    else:
                        j = t - T_tail
                        nc.vector.tensor_reduce(
                            out=Mx4_sb[:, mi, NTC + j : NTC + j + 1], in_=ps4,
                            axis=AXXY, op=ALU_MAX,
                        )

            # ---------------- interleaved pipeline ----------------
            # B order: pure tiles (T_tail..TM-1) then tail tiles (0..T_tail-1)
            bseq = list(range(T_tail, TM)) + list(range(T_tail))

            def need_a(bj):
                t = bseq[bj]
                if t < T_tail:
                    return TM  # tail B needs every slot's g (UsT)
                j = t - T_tail
                return max(bounds[shi_pure[j]], t + 1)

            LEAD = max(max(need_a(j) - j for j in range(TM)) + 1, need_a(0))

            a_next = 0
            b_next = 0
            l3_next = 0
            g_emitted = [False] * S
            ust_emitted = False
            ub_emitted = [False] * P_pure
            dmad = 0

            def try_unlock():
                nonlocal ust_emitted
                for s in range(S):
                    if not g_emitted[s] and a_next >= bounds[s]:
                        emit_g(s)
                        g_emitted[s] = True
                if not ust_emitted and all(g_emitted):
                    emit_UsT()
                    ust_emitted = True

            def b_ready(bj):
                t = bseq[bj]
                if t < T_tail:
                    return ust_emitted
                j = t - T_tail
                return g_emitted[shi_pure[j]] and a_next > t

            while b_next < TM:
                while a_next < min(TM, b_next + LEAD):
                    emit_A(a_next)
                    a_next += 1
                    try_unlock()
                progressed = False
                while l3_next <= min(b_next + 1, TM - 1) and b_ready(l3_next):
                    t = bseq[l3_next]
                    if t >= T_tail:
                        # pace Ub builds: this tile now, next one ahead
                        for jj in (t - T_tail, t - T_tail + 1):
                            if (
                                0 <= jj < P_pure
                                and not ub_emitted[jj]
                                and g_emitted[shi_pure[jj]]
                            ):
                                emit_Ub(jj)
                                ub_emitted[jj] = True
                    emit_L3(t)
                    l3_next += 1
                    progressed = True
                if l3_next > b_next:
                    emit_L4(bseq[b_next])
                    b_next += 1
                    progressed = True
                    # stream completed pure columns out while computing
                    if b_next in (10, 20) and bseq[b_next - 1] >= T_tail:
                        j1 = bseq[b_next - 1] - T_tail + 1
                        nc.sync.dma_start(
                            out=mx4.ap()[:, :, NTC + dmad : NTC + j1],
                            in_=Mx4_sb[:, :, NTC + dmad : NTC + j1],
                        )
                        dmad = j1
                if not progressed:
                    if a_next < TM:
                        emit_A(a_next)
                        a_next += 1
                        try_unlock()
                    else:
                        raise RuntimeError("pipeline deadlock")

            nc.sync.dma_start(
                out=mx4.ap()[:, :, NTC + dmad : GW], in_=Mx4_sb[:, :, NTC + dmad : GW]
            )
            nc.sync.dma_start(out=mx4.ap()[:, :, 0:NTC], in_=Mx4_sb[:, :, 0:NTC])

    nc.finalize()
    return nc


def _partition(npts: np.ndarray, n_cores: int, slots: int):
    """Assign whole segments to cores, balancing total points.

    Returns per-core segment-id lists (each <= slots long).
    """
    B = len(npts)
    order = np.argsort(-npts, kind="stable")
    best = None
    for trial in range(64):
        rng = np.random.default_rng(trial)
        seq = order.copy() if trial == 0 else rng.permutation(B)
        seq = sorted(seq, key=lambda s: -npts[s])
        if trial > 0:  # tie-break shuffles
            k = trial % 4 + 1
            seq = list(seq)
            for i in range(0, len(seq) - k, k):
                sub = seq[i : i + k]
                rng.shuffle(sub)
                seq[i : i + k] = sub
        groups = [[] for _ in range(n_cores)]
        loads = [0] * n_cores
        for s in seq:
            cands = [c for c in range(n_cores) if len(groups[c]) < slots]
            c = min(cands, key=lambda i: loads[i])
            groups[c].append(int(s))
            loads[c] += int(npts[s])
        for _ in range(400):
            hi = max(range(n_cores), key=lambda i: loads[i])
            done = True
            for lo in sorted(range(n_cores), key=lambda i: loads[i]):
                if lo == hi:
                    continue
                for ia, sa in enumerate(groups[hi]):
                    for ib, sb in enumerate(groups[lo]):
                        d = int(npts[sa]) - int(npts[sb])
                        if d > 0 and max(loads[hi] - d, loads[lo] + d) < loads[hi]:
                            groups[hi][ia], groups[lo][ib] = sb, sa
                            loads[hi] -= d
                            loads[lo] += d
                            done = False
                            break
                    if not done:
                        break
                if not done:
                    break
            if done:
                break
        key = (max(loads), tuple(sorted(loads)))
        if best is None or key < best[0]:
            best = (key, [list(g) for g in groups])
    return best[1]


def _prepare(x: np.ndarray, seg_ids: np.ndarray, B: int):
    counts = np.bincount(seg_ids, minlength=B)
    starts = np.concatenate([[0], np.cumsum(counts)])
    S = (B + N_CORES - 1) // N_CORES

    groups = _partition(counts.astype(np.int64), N_CORES, S)

    # per-core structure: order segments smallest-first (slot 0 smallest so
    # phase B unlocks early); demote full 512-blocks of the last slot into
    # the tail region when a core has more full tiles than P_pure.
    for c in range(N_CORES):
        groups[c] = sorted(groups[c], key=lambda s: counts[s])

    fulls = [sum(int(counts[s]) // PT for s in g) for g in groups]
    rem64 = [
        sum(-(-(int(counts[s]) % PT) // GRP) * GRP for s in g if int(counts[s]) % PT)
        for g in groups
    ]

    best = None
    for P_pure in range(max(1, min(fulls) - 2), max(fulls) + 1):
        T_tail = 0
        ok = True
        for c in range(N_CORES):
            tail_pts = rem64[c] + max(0, fulls[c] - P_pure) * PT
            T_tail = max(T_tail, -(-tail_pts // PT))
        TM = P_pure + T_tail
        if best is None or (TM, T_tail) < best[:2]:
            best = (TM, T_tail, P_pure)
    TM, T_tail, P_pure = best
    T_tail = max(T_tail, 1)
    TM = T_tail + P_pure
    NTC = T_tail * NG
    GW = NTC + P_pure

    xT_cores, gmask_cores, umaskT_cores, onehot_cores, post = [], [], [], [], []
    pure_slots_all = []
    for c in range(N_CORES):
        segs = groups[c]
        demote = max(0, fulls[c] - P_pure)
        # per-slot: full blocks (pure tiles) and tail chunk points
        pure_blocks = []  # (slot, pts[512,3])
        tail_parts = []  # (slot, pts[n64,3])
        for k, s in enumerate(segs):
            pts = x[starts[s] : starts[s + 1]]
            nf = len(pts) // PT
            rem = pts[nf * PT :]
            dem = demote if k == len(segs) - 1 else 0
            dem = min(dem, nf)
            nf -= dem
            tail_pts = [pts[(nf + d) * PT : (nf + d + 1) * PT] for d in range(dem)]
            if len(rem):
                pad = -len(rem) % GRP
                rem = np.concatenate([rem, rem[: pad]]) if pad else rem
                while len(rem) % GRP:  # pad smaller than remainder handled above
                    rem = np.concatenate([rem, rem[: GRP - len(rem) % GRP]])
                tail_pts.append(rem)
            for d in range(nf):
                pure_blocks.append((k, pts[d * PT : (d + 1) * PT]))
            if tail_pts:
                tail_parts.append((k, np.concatenate(tail_pts)))
        # pure padding: duplicate the last pure block (its slot label kept)
        while len(pure_blocks) < P_pure:
            pure_blocks.append(pure_blocks[-1])
        assert len(pure_blocks) == P_pure, (c, len(pure_blocks), P_pure)
        # tail assembly
        if not tail_parts:
            k = len(segs) - 1
            tail_parts.append((k, np.tile(x[starts[segs[k]] : starts[segs[k]] + 1], (GRP, 1))))
        tail_slots_pts = np.concatenate([p for _, p in tail_parts])
        tail_grp_slot = sum(([k] * (len(p) // GRP) for k, p in tail_parts), [])
        need = T_tail * PT - len(tail_slots_pts)
        assert need >= 0, (c, len(tail_slots_pts))
        if need:
            lastk, lastp = tail_parts[-1]
            reps = np.tile(lastp[: GRP], (need // GRP, 1))
            tail_slots_pts = np.concatenate([tail_slots_pts, reps])
            tail_grp_slot += [lastk] * (need // GRP)
        assert len(tail_grp_slot) == NTC

        pure_slots = [k for k, _ in pure_blocks]
        pure_slots_all.append(pure_slots)

        xc = np.concatenate([tail_slots_pts] + [p for _, p in pure_blocks]).astype(np.float16)
        xT_cores.append(np.ascontiguousarray(xc.T))

        gm = np.zeros((S, GW), np.float32)
        for col, k in enumerate(tail_grp_slot):
            gm[k, col] = 1.0
        for j, k in enumerate(pure_slots):
            gm[k, NTC + j] = 1.0
        gmask_cores.append(np.ascontiguousarray(np.broadcast_to(gm[None], (128, S, GW))))

        um = np.zeros((P_pure, S), np.float32)
        for j, k in enumerate(pure_slots):
            um[j, k] = 1.0
        umaskT_cores.append(np.ascontiguousarray(np.broadcast_to(um[None], (128, P_pure, S))))

        oh = np.zeros((S, T_tail * PT), np.float16)
        for gcol, k in enumerate(tail_grp_slot):
            oh[k, gcol * GRP : (gcol + 1) * GRP] = 1.0
        onehot_cores.append(oh)

        post.append((segs, pure_slots, tail_grp_slot))

    bounds = tuple(
        T_tail + max(
            (max((j + 1 for j, k in enumerate(psl) if k <= s), default=0))
            for psl in pure_slots_all
        )
        for s in range(S)
    )
    shi_pure = tuple(
        max(pure_slots_all[c][j] for c in range(N_CORES)) for j in range(P_pure)
    )
    return (
        (T_tail, P_pure, S, bounds, shi_pure),
        xT_cores, gmask_cores, umaskT_cores, onehot_cores, post,
    )


def make_in_maps(inputs):
    x = np.asarray(inputs["x"], np.float32)
    seg_ids = np.asarray(inputs["seg_ids"])
    B = int(inputs["num_segments"])

    Wf, bf = [], []
    for i in (1, 2, 3, 4):
        W = np.asarray(inputs[f"W{i}"], np.float32)
        b = np.asarray(inputs[f"b{i}"], np.float32)
        ga = np.asarray(inputs[f"g{i}"], np.float32)
        be = np.asarray(inputs[f"be{i}"], np.float32)
        m = np.asarray(inputs[f"m{i}"], np.float32)
        v = np.asarray(inputs[f"v{i}"], np.float32)
        sc = ga / np.sqrt(v + EPS)
        Wf.append(np.ascontiguousarray(W * sc[None, :]))
        bf.append((b - m) * sc + be)
    W1f, W2f, W3f, W4f = Wf
    b1f, b2f, b3f, b4f = bf

    key, xT_cores, gmask_cores, umaskT_cores, onehot_cores, post = _prepare(x, seg_ids, B)

    w1d = W1f.astype(np.float16)
    w2d = np.ascontiguousarray(W2f.reshape(128, 2, 128).astype(np.float16))
    w3ad = np.ascontiguousarray(
        W3f[:256].reshape(2, 128, 4, 128).transpose(1, 0, 2, 3).astype(np.float16)
    )
    w3bd = np.ascontiguousarray(
        W3f[256:].reshape(2, 128, 4, 128).transpose(1, 0, 2, 3).astype(np.float16)
    )
    w4d = np.ascontiguousarray(
        W4f.reshape(4, 128, 8, 128).transpose(1, 0, 2, 3).astype(np.float16)
    )
    b1d = np.ascontiguousarray(b1f.reshape(128, 1))
    b2d = np.ascontiguousarray(b2f.reshape(2, 128).T)
    b3d = np.ascontiguousarray(b3f.reshape(4, 128).T)

    in_maps = [
        {
            "xT": xT_cores[c],
            "gmask": gmask_cores[c],
            "umaskT": umaskT_cores[c],
            "onehot": onehot_cores[c],
            "w1": w1d,
            "w2": w2d,
            "w3a": w3ad,
            "w3b": w3bd,
            "w4": w4d,
            "b1": b1d,
            "b2": b2d,
            "b3": b3d,
        }
        for c in range(N_CORES)
    ]
    return key, in_maps, post, b4f


def postprocess(results, post, b4f, B, T_tail, P_pure):
    NTC = T_tail * NG
    out = np.zeros((B, 1024), np.float32)
    for c in range(N_CORES):
        mx4 = results[c]["mx4"]  # [128, 8, GW]
        segs, pure_slots, tail_grp_slot = post[c]
        for k, s in enumerate(segs):
            cols = [g for g, kk in enumerate(tail_grp_slot) if kk == k]
            cols += [NTC + j for j, kk in enumerate(pure_slots) if kk == k]
            raw = mx4[:, :, cols].max(axis=2)  # [128, 8]
            out[s] = np.maximum(raw.T.reshape(1024) + b4f, 0.0)
    return out


def get_program(key):
    if key not in _PROGRAM_CACHE:
        _PROGRAM_CACHE[key] = _build_program(*key)
    return _PROGRAM_CACHE[key]


def kernel(**inputs) -> np.ndarray:
    B = int(inputs["num_segments"])
    key, in_maps, post, b4f = make_in_maps(inputs)
    nc = get_program(key)
    last_err = None
    for _ in range(3):  # retry transient NRT device wedges
        try:
            res = run_bass_kernel_spmd(nc, in_maps, core_ids=list(range(N_CORES)))
            return postprocess(res.results, post, b4f, B, key[0], key[1])
        except Exception as e:  # noqa: BLE001
            last_err = e
    raise last_err


# revision 5
# speedup vs baseline: 1.0404x; 1.0059x over previous
"""PointNet-style encoder (conv1x1 stack + ragged segment-max) on 8 Trainium2 cores.

Strategy (v2.1 — tail-tile packing)
-----------------------------------
* BN folded into conv weights host-side; every layer is matmul+bias+ReLU.
* Feature-major on device: activations live as [C, points] tiles; points stream
  through the PE as the matmul free dimension in 512-point macro-tiles.
* Segments are point-balanced across the 8 cores (whole segments per core, so
  the two segment-maxes stay core-local). Per core the layout is:
    - T_TAIL fixed "tail" tiles (array idx 0..T_TAIL-1) holding every
      segment's sub-512 remainder, packed as 64-col-aligned per-slot chunks;
    - P_PURE "pure" tiles (idx T_TAIL..) each belonging to one segment.
  This wastes ~1 tile/core instead of the ~3 of per-segment padding.
* Phase A (L1+L2) runs tail tiles first, then pures in slot order; per-tile
  f2 maxes (Mx2) land in a combined column buffer: 8 per-64-col group maxes
  per tail tile, 1 per pure tile. Per-slot g = masked max over that buffer
  (gmask input zeros other slots; buffer memset 0 + f2>=0 keeps unwritten
  columns neutral).
* Mid-network unpool via concat identity: concat(f2, g)@W3 = f2@W3a + g@W3b.
  Per slot Us = W3b^T g + b3 (tiny 1-col matmuls). Pure tiles read a per-tile
  bias column Ub[:,m,j], built by one masked broadcast pass per slot (umask2
  input). Tail tiles get the g-term exactly via an extra accumulated matmul
  per m-chunk: lhsT = UsT (computed on-PE as G2^T@W3b, [S,512]), rhs =
  one-hot slot matrix O [S, cols] — per-point unpool without masks.
* Phase B (L3+L4) runs pures first (unlocked per slot as g arrives), tail
  tiles last (they need every slot's UsT). L4 maxes reduce per pure tile and
  per 64-col group for tail tiles (raw, pre-bias); the host applies
  relu(.+b4) and combines columns per segment (exact under max).
* Single interleaved pipeline (phase A runs LEAD tiles ahead of phase B) so
  A's ACT/DVE drains hide under B's PE-bound matmuls; dummy matmuls keep the
  PE HAM clock-gate open through the drain-paced fill phase.
* Matmuls in float16 (fp32 PSUM): 1 cycle/column, ~1e-3 rel err.
"""

import numpy as np

import concourse.bass as bass
import concourse.mybir as mybir
import concourse.tile as tile
from concourse import bacc
from concourse.bass_utils import run_bass_kernel_spmd

N_CORES = 8
PT = 512
GRP = 64  # tail group granularity (cols)
NG = PT // GRP  # groups per tile
EPS = 1e-3  # keras BatchNormalization default epsilon

F32 = mybir.dt.float32
F16 = mybir.dt.float16
AF = mybir.ActivationFunctionType
AXX = mybir.AxisListType.X
AXXY = mybir.AxisListType.XY
ALU_MAX = mybir.AluOpType.max
ALU_ADD = mybir.AluOpType.add

_PROGRAM_CACHE: dict = {}


def _build_program(T_tail: int, P_pure: int, S: int, bounds: tuple, shi_pure: tuple):
    """One SPMD program for all cores.

    bounds[s]: #A-tiles after which slot s's f2 is complete on every core.
    shi_pure[j]: max (over cores) slot id of pure tile j.
    """
    TM = T_tail + P_pure
    NTC = T_tail * NG  # tail group columns
    GW = NTC + P_pure  # combined max-buffer width

    nc = bacc.Bacc("TRN2")

    xT = nc.dram_tensor("xT", [3, TM * PT], F16, kind="ExternalInput")
    gmask = nc.dram_tensor("gmask", [128, S, GW], F32, kind="ExternalInput")
    umask2 = nc.dram_tensor("umask2", [128, S, P_pure], F32, kind="ExternalInput")
    onehot = nc.dram_tensor("onehot", [S, T_tail * PT], F16, kind="ExternalInput")
    w1 = nc.dram_tensor("w1", [3, 128], F16, kind="ExternalInput")
    w2 = nc.dram_tensor("w2", [128, 2, 128], F16, kind="ExternalInput")
    w3a = nc.dram_tensor("w3a", [128, 2, 4, 128], F16, kind="ExternalInput")
    w3b = nc.dram_tensor("w3b", [128, 2, 4, 128], F16, kind="ExternalInput")
    w4 = nc.dram_tensor("w4", [128, 4, 8, 128], F16, kind="ExternalInput")
    b1 = nc.dram_tensor("b1", [128, 1], F32, kind="ExternalInput")
    b2 = nc.dram_tensor("b2", [128, 2], F32, kind="ExternalInput")
    b3 = nc.dram_tensor("b3", [128, 4], F32, kind="ExternalInput")
    mx4 = nc.dram_tensor("mx4", [128, 8, GW], F32, kind="ExternalOutput")

    X_HEAD = min(6, TM)  # tiles in the first (prioritized) x DMA

    with tile.TileContext(nc) as tc:
        with (
            tc.tile_pool(name="const", bufs=1) as constp,
            tc.tile_pool(name="h1p", bufs=3) as h1p,
            tc.tile_pool(name="h3p", bufs=3) as h3p,
            tc.tile_pool(name="tmpp", bufs=4) as tmpp,
            tc.tile_pool(name="psA", bufs=4, space="PSUM") as psA,
            tc.tile_pool(name="psB3", bufs=2, space="PSUM") as psB3,
            tc.tile_pool(name="psB4", bufs=2, space="PSUM") as psB4,
        ):
            # x head first on the sync queue so the first L1 unblocks ASAP
            x_sb = constp.tile([3, TM * PT], F16)
            nc.sync.dma_start(out=x_sb[:, : X_HEAD * PT], in_=xT.ap()[:, : X_HEAD * PT])
            w1_sb = constp.tile([3, 128], F16)
            nc.sync.dma_start(out=w1_sb, in_=w1.ap())
            w2_sb = constp.tile([128, 2, 128], F16)
            nc.sync.dma_start(out=w2_sb, in_=w2.ap())
            b1_sb = constp.tile([128, 1], F32)
            nc.sync.dma_start(out=b1_sb, in_=b1.ap())
            b2_sb = constp.tile([128, 2], F32)
            nc.sync.dma_start(out=b2_sb, in_=b2.ap())
            b3_sb = constp.tile([128, 4], F32)
            nc.sync.dma_start(out=b3_sb, in_=b3.ap())
            nc.sync.dma_start(out=x_sb[:, X_HEAD * PT :], in_=xT.ap()[:, X_HEAD * PT :])
            # phase-B weights + masks on the gpsimd queue, in need order
            w3a_sb = constp.tile([128, 2, 4, 128], F16)
            nc.gpsimd.dma_start(out=w3a_sb, in_=w3a.ap())
            gmask_sb = constp.tile([128, S, GW], F32)
            nc.gpsimd.dma_start(out=gmask_sb, in_=gmask.ap())
            umask2_sb = constp.tile([128, S, P_pure], F32)
            nc.gpsimd.dma_start(out=umask2_sb, in_=umask2.ap())
            w3b_sb = constp.tile([128, 2, 4, 128], F16)
            nc.gpsimd.dma_start(out=w3b_sb, in_=w3b.ap())
            w4_sb = constp.tile([128, 4, 8, 128], F16)
            nc.gpsimd.dma_start(out=w4_sb, in_=w4.ap())
            onehot_sb = constp.tile([S, T_tail * PT], F16)
            nc.gpsimd.dma_start(out=onehot_sb, in_=onehot.ap())

            # f2 storage: tail tiles keep a grouped view for 64-col reduces
            f2_tail = constp.tile([128, T_tail, 2, NG, GRP], F16)
            f2_pure = constp.tile([128, P_pure, 2, PT], F16)
            Mx2_sb = constp.tile([128, 2, GW], F32)
            g_sb = constp.tile([128, 2, S], F32)
            G2s_sb = constp.tile([128, 2, S], F16)
            Us_sb = constp.tile([128, 4, S], F32)
            Ub_sb = constp.tile([128, 4, P_pure], F32)
            UsT_sb = constp.tile([S, 4, 128], F16)
            Mx4_sb = constp.tile([128, 8, GW], F32)

            # zero-init buffers that masked ops may read before fully written
            nc.vector.memset(Mx2_sb, 0.0)
            nc.vector.memset(Us_sb, 0.0)
            nc.vector.memset(G2s_sb, 0.0)

            # HAM warmup: dependency-free matmuls during the DMA prologue
            warm_src = constp.tile([128, PT], F16, name="warm_src")
            nc.vector.memset(warm_src, 0.01)
            warm_out = constp.tile([128, 1], F32, name="warm_out")
            warm_act = constp.tile([128, 8], F32, name="warm_act")
            nc.scalar.activation(out=warm_act, in_=warm_src[:, 0:8], func=AF.Relu)
            ps_dummy = psB4.tile([128, PT], F32, tag="ps4", name="ps_warm")
            for _ in range(8):
                nc.tensor.matmul(
                    ps_dummy[:, :], warm_src[:, 0:128], warm_src[:, :], start=True, stop=True
                )
            nc.vector.tensor_reduce(out=warm_out, in_=ps_dummy[:, 0:8], axis=AXX, op=ALU_MAX)

            def f2v(t):
                return f2_tail[:, t] if t < T_tail else f2_pure[:, t - T_tail]

            deferred_mx2: list = []

            def emit_mx2(t):
                if t < T_tail:
                    nc.vector.tensor_reduce(
                        out=Mx2_sb[:, :, t * NG : (t + 1) * NG], in_=f2_tail[:, t],
                        axis=AXX, op=ALU_MAX,
                    )
                else:
                    j = t - T_tail
                    nc.vector.tensor_reduce(
                        out=Mx2_sb[:, :, NTC + j : NTC + j + 1], in_=f2_pure[:, j],
                        axis=AXX, op=ALU_MAX,
                    )

            def emit_A(t, fill):
                """L1+L2 for array tile t; stores f2 (fp16) + its Mx2 column(s)."""
                tail = t < T_tail
                shp = [128, NG, GRP] if tail else [128, PT]
                ps1 = psA.tile(shp, F32, tag="psa", name=f"ps1_{t}")
                nc.tensor.matmul(
                    ps1[:, :], w1_sb[:, :], x_sb[:, t * PT : (t + 1) * PT],
                    start=True, stop=True,
                )
                h1_sb = h1p.tile(shp, F16, tag="h1", name=f"h1_{t}")
                nc.scalar.activation(out=h1_sb, in_=ps1, func=AF.Relu, bias=b1_sb[:, 0:1])
                for c in range(2):
                    ps2 = psA.tile(shp, F32, tag="psa", name=f"ps2_{t}_{c}")
                    nc.tensor.matmul(ps2[:, :], w2_sb[:, c, :], h1_sb[:, :], start=True, stop=True)
                    # engine split: fill is drain-paced, steady state hides
                    # drains under phase B; Mx2 for pre-bounds[0] tiles must
                    # run inline (slot 0 unlock), later fill tiles defer
                    if fill and t >= bounds[0]:
                        on_dve = True  # both chunks on DVE, no Mx2 inline
                    else:
                        on_dve = c == 1 and fill
                    if not fill:
                        on_dve = False  # steady: ACT takes all three drains
                    if on_dve:
                        nc.vector.tensor_scalar(
                            f2v(t)[:, c], ps2, b2_sb[:, c : c + 1], 0.0, ALU_ADD, ALU_MAX
                        )
                    else:
                        nc.scalar.activation(
                            out=f2v(t)[:, c], in_=ps2, func=AF.Relu, bias=b2_sb[:, c : c + 1]
                        )
                if fill and t >= bounds[0]:
                    deferred_mx2.append(t)
                else:
                    emit_mx2(t)

            def emit_g(s):
                """g[s] = masked max over the Mx2 buffer; then Us[s], Ub pass."""
                for c in range(2):
                    tmp = tmpp.tile([128, GW], F32, tag="tmp", name=f"tmpg_{c}_{s}")
                    nc.vector.tensor_mul(tmp[:, :], Mx2_sb[:, c, :], gmask_sb[:, s, :])
                    nc.vector.tensor_reduce(
                        out=g_sb[:, c, s : s + 1], in_=tmp[:, :], axis=AXX, op=ALU_MAX
                    )
                nc.scalar.copy(G2s_sb[:, :, s], g_sb[:, :, s])
                psu = psA.tile([128, PT], F32, tag="psa", name=f"psu_{s}")
                for m in range(4):
                    nc.tensor.matmul(
                        psu[:, m : m + 1], w3b_sb[:, 0, m, :], G2s_sb[:, 0, s : s + 1],
                        start=True, stop=False,
                    )
                    nc.tensor.matmul(
                        psu[:, m : m + 1], w3b_sb[:, 1, m, :], G2s_sb[:, 1, s : s + 1],
                        start=False, stop=True,
                    )
                for m in range(4):
                    nc.scalar.activation(
                        out=Us_sb[:, m, s : s + 1], in_=psu[:, m : m + 1],
                        func=AF.Identity, bias=b3_sb[:, m : m + 1],
                    )
                # broadcast Us[slot] into the per-tile bias table
                for m in range(4):
                    if s == 0:
                        nc.vector.tensor_scalar_mul(
                            Ub_sb[:, m, :], umask2_sb[:, s, :], Us_sb[:, m, s : s + 1]
                        )
                    else:
                        tmpu = tmpp.tile([128, P_pure], F32, tag="tmpu", name=f"tmpu_{s}_{m}")
                        nc.vector.tensor_scalar_mul(
                            tmpu[:, :], umask2_sb[:, s, :], Us_sb[:, m, s : s + 1]
                        )
                        nc.vector.tensor_add(Ub_sb[:, m, :], Ub_sb[:, m, :], tmpu[:, :])

            def emit_UsT():
                """UsT = (G2s)^T @ W3b  -> [S, 512] fp16 (for tail unpool)."""
                ps = psB3.tile([128, 4, 128], F32, tag="ps3", name="ps_ust")
                nc.tensor.matmul(
                    ps[0:S, :, :], G2s_sb[:, 0, 0:S], w3b_sb[:, 0], start=True, stop=False
                )
                nc.tensor.matmul(
                    ps[0:S, :, :], G2s_sb[:, 1, 0:S], w3b_sb[:, 1], start=False, stop=True
                )
                nc.scalar.copy(UsT_sb[0:S], ps[0:S, :, :])

            h3_tiles = {}

            def emit_L3(t):
                tail = t < T_tail
                h3_sb = h3p.tile([128, 4, PT], F16, tag="h3", name=f"h3_{t}")
                for m in range(4):
                    ps3 = psB3.tile([128, PT], F32, tag="ps3", name=f"ps3_{t}_{m}")
                    nc.tensor.matmul(
                        ps3[:, :], w3a_sb[:, 0, m, :], f2v(t)[:, 0], start=True, stop=False
                    )
                    nc.tensor.matmul(
                        ps3[:, :], w3a_sb[:, 1, m, :], f2v(t)[:, 1],
                        start=False, stop=not tail,
                    )
                    if tail:
                        nc.tensor.matmul(
                            ps3[:, :], UsT_sb[0:S, m, :],
                            onehot_sb[0:S, t * PT : (t + 1) * PT],
                            start=False, stop=True,
                        )
                        bias = b3_sb[:, m : m + 1]
                    else:
                        j = t - T_tail
                        bias = Ub_sb[:, m, j : j + 1]
                    nc.scalar.activation(out=h3_sb[:, m], in_=ps3, func=AF.Relu, bias=bias)
                h3_tiles[t] = h3_sb

            def emit_L4(t):
                tail = t < T_tail
                h3_sb = h3_tiles.pop(t)
                for mi in range(8):
                    shp = [128, NG, GRP] if tail else [128, PT]
                    ps4 = psB4.tile(shp, F32, tag="ps4", name=f"ps4_{t}_{mi}")
                    for k in range(4):
                        nc.tensor.matmul(
                            ps4[:, :], w4_sb[:, k, mi, :], h3_sb[:, k],
                            start=(k == 0), stop=(k == 3),
                        )
                    if tail:
                        nc.vector.tensor_reduce(
                            out=Mx4_sb[:, mi, t * NG : (t + 1) * NG], in_=ps4,
                            axis=AXX, op=ALU_MAX,
                        )
                    else:
                        j = t - T_tail
                        nc.vector.tensor_reduce(
                            out=Mx4_sb[:, mi, NTC + j : NTC + j + 1], in_=ps4,
                            axis=AXX, op=ALU_MAX,
                        )

            # ---------------- interleaved pipeline ----------------
            # B order: pure tiles (T_tail..TM-1) then tail tiles (0..T_tail-1)
            bseq = list(range(T_tail, TM)) + list(range(T_tail))

            def need_a(bj):
                t = bseq[bj]
                if t < T_tail:
                    return TM  # tail B needs every slot's g (UsT)
                j = t - T_tail
                return max(bounds[shi_pure[j]], t + 1)

            LEAD = max(max(need_a(j) - j for j in range(TM)) + 1, need_a(0))

            a_next = 0
            b_next = 0
            l3_next = 0
            g_emitted = [False] * S
            ust_emitted = False
            dmad = 0

            def try_unlock():
                nonlocal ust_emitted
                for s in range(S):
                    if not g_emitted[s] and a_next >= bounds[s]:
                        for t in [d for d in deferred_mx2 if d < bounds[s]]:
                            emit_mx2(t)
                            deferred_mx2.remove(t)
                        emit_g(s)
                        g_emitted[s] = True
                if not ust_emitted and all(g_emitted):
                    emit_UsT()
                    ust_emitted = True

            def b_ready(bj):
                t = bseq[bj]
                if t < T_tail:
                    return ust_emitted
                return g_emitted[shi_pure[t - T_tail]] and a_next > t

            while b_next < TM:
                while a_next < min(TM, b_next + LEAD):
                    emit_A(a_next, fill=(b_next == 0))
                    if b_next == 0:
                        # dummy matmuls keep the HAM clock gate open through
                        # the drain-paced fill
                        for _ in range(2):
                            nc.tensor.matmul(
                                ps_dummy[:, :PT], warm_src[:, 0:128], warm_src[:, :],
                                start=True, stop=True,
                            )
                    a_next += 1
                    try_unlock()
                progressed = False
                while l3_next <= min(b_next + 1, TM - 1) and b_ready(l3_next):
                    emit_L3(bseq[l3_next])
                    l3_next += 1
                    progressed = True
                if l3_next > b_next:
                    if b_next == 0:
                        # bridge the prologue stall (L4(0) waits on first h3)
                        for _ in range(8):
                            nc.tensor.matmul(
                                ps_dummy[:, :PT], warm_src[:, 0:128], warm_src[:, :],
                                start=True, stop=True,
                            )
                    # flush one deferred Mx2 per B tile into steady-state slack
                    if deferred_mx2:
                        emit_mx2(deferred_mx2.pop(0))
                    emit_L4(bseq[b_next])
                    b_next += 1
                    progressed = True
                    # stream completed pure columns out while computing
                    if b_next in (10, 20):
                        nc.sync.dma_start(
                            out=mx4.ap()[:, :, NTC + dmad : NTC + b_next],
                            in_=Mx4_sb[:, :, NTC + dmad : NTC + b_next],
                        )
                        dmad = b_next
                if not progressed:
                    if a_next < TM:
                        emit_A(a_next, fill=(b_next == 0))
                        a_next += 1
                        try_unlock()
                    else:
                        raise RuntimeError("pipeline deadlock")

            nc.sync.dma_start(
                out=mx4.ap()[:, :, NTC + dmad : GW], in_=Mx4_sb[:, :, NTC + dmad : GW]
            )
            nc.sync.dma_start(out=mx4.ap()[:, :, 0:NTC], in_=Mx4_sb[:, :, 0:NTC])

    nc.finalize()
    return nc


def _partition(npts: np.ndarray, n_cores: int, slots: int):
    """Assign whole segments to cores, balancing total points."""
    B = len(npts)
    order = np.argsort(-npts, kind="stable")
    best = None
    for trial in range(64):
        rng = np.random.default_rng(trial)
        seq = order.copy() if trial == 0 else rng.permutation(B)
        seq = sorted(seq, key=lambda s: -npts[s])
        if trial > 0:  # tie-break shuffles
            k = trial % 4 + 1
            seq = list(seq)
            for i in range(0, len(seq) - k, k):
                sub = seq[i : i + k]
                rng.shuffle(sub)
                seq[i : i + k] = sub
        groups = [[] for _ in range(n_cores)]
        loads = [0] * n_cores
        for s in seq:
            cands = [c for c in range(n_cores) if len(groups[c]) < slots]
            c = min(cands, key=lambda i: loads[i])
            groups[c].append(int(s))
            loads[c] += int(npts[s])
        for _ in range(400):
            hi = max(range(n_cores), key=lambda i: loads[i])
            done = True
            for lo in sorted(range(n_cores), key=lambda i: loads[i]):
                if lo == hi:
                    continue
                for ia, sa in enumerate(groups[hi]):
                    for ib, sb in enumerate(groups[lo]):
                        d = int(npts[sa]) - int(npts[sb])
                        if d > 0 and max(loads[hi] - d, loads[lo] + d) < loads[hi]:
                            groups[hi][ia], groups[lo][ib] = sb, sa
                            loads[hi] -= d
                            loads[lo] += d
                            done = False
                            break
                    if not done:
                        break
                if not done:
                    break
            if done:
                break
        key = (max(loads), tuple(sorted(loads)))
        if best is None or key < best[0]:
            best = (key, [list(g) for g in groups])
    return best[1]


def _prepare(x: np.ndarray, seg_ids: np.ndarray, B: int):
    counts = np.bincount(seg_ids, minlength=B)
    starts = np.concatenate([[0], np.cumsum(counts)])
    S = (B + N_CORES - 1) // N_CORES

    groups = _partition(counts.astype(np.int64), N_CORES, S)
    # slot 0 smallest so phase B unlocks early
    for c in range(N_CORES):
        groups[c] = sorted(groups[c], key=lambda s: counts[s])

    fulls = [sum(int(counts[s]) // PT for s in g) for g in groups]
    rem64 = [
        sum(-(-(int(counts[s]) % PT) // GRP) * GRP for s in g if int(counts[s]) % PT)
        for g in groups
    ]

    best = None
    for P_pure in range(max(1, min(fulls) - 2), max(fulls) + 1):
        T_tail = 1
        for c in range(N_CORES):
            tail_pts = rem64[c] + max(0, fulls[c] - P_pure) * PT
            T_tail = max(T_tail, -(-tail_pts // PT))
        TM = P_pure + T_tail
        if best is None or (TM, T_tail) < best[:2]:
            best = (TM, T_tail, P_pure)
    TM, T_tail, P_pure = best
    NTC = T_tail * NG
    GW = NTC + P_pure

    xT_cores, gmask_cores, umask2_cores, onehot_cores, post = [], [], [], [], []
    pure_slots_all = []
    for c in range(N_CORES):
        segs = groups[c]
        demote = max(0, fulls[c] - P_pure)
        pure_blocks = []  # (slot, pts[512,3])
        tail_parts = []  # (slot, pts[n*64,3])
        for k, s in enumerate(segs):
            pts = x[starts[s] : starts[s + 1]]
            nf = len(pts) // PT
            rem = pts[nf * PT :]
            if len(rem):
                while len(rem) % GRP:
                    rem = np.concatenate([rem, rem[: GRP - len(rem) % GRP]])
                tail_parts.append((k, rem))
            for d in range(nf):
                pure_blocks.append((k, pts[d * PT : (d + 1) * PT]))
        # demote full blocks (from the end = largest slots) into the tail
        for _ in range(demote):
            k, blk = pure_blocks.pop()
            tail_parts.append((k, blk))
        while len(pure_blocks) < P_pure:
            pure_blocks.append(pure_blocks[-1])
        assert len(pure_blocks) == P_pure, (c, len(pure_blocks), P_pure)
        if not tail_parts:
            k = len(segs) - 1
            tail_parts.append(
                (k, np.tile(x[starts[segs[k]] : starts[segs[k]] + 1], (GRP, 1)))
            )
        tail_pts = np.concatenate([p for _, p in tail_parts])
        tail_grp_slot = sum(([k] * (len(p) // GRP) for k, p in tail_parts), [])
        need = T_tail * PT - len(tail_pts)
        assert need >= 0, (c, len(tail_pts))
        if need:
            lastk = tail_parts[-1][0]
            reps = np.tile(tail_parts[-1][1][:GRP], (need // GRP, 1))
            tail_pts = np.concatenate([tail_pts, reps])
            tail_grp_slot += [lastk] * (need // GRP)
        assert len(tail_grp_slot) == NTC

        pure_slots = [k for k, _ in pure_blocks]
        pure_slots_all.append(pure_slots)

        xc = np.concatenate([tail_pts] + [p for _, p in pure_blocks]).astype(np.float16)
        xT_cores.append(np.ascontiguousarray(xc.T))

        gm = np.zeros((S, GW), np.float32)
        for col, k in enumerate(tail_grp_slot):
            gm[k, col] = 1.0
        for j, k in enumerate(pure_slots):
            gm[k, NTC + j] = 1.0
        gmask_cores.append(np.ascontiguousarray(np.broadcast_to(gm[None], (128, S, GW))))

        um = np.zeros((S, P_pure), np.float32)
        for j, k in enumerate(pure_slots):
            um[k, j] = 1.0
        umask2_cores.append(
            np.ascontiguousarray(np.broadcast_to(um[None], (128, S, P_pure)))
        )

        oh = np.zeros((S, T_tail * PT), np.float16)
        for gcol, k in enumerate(tail_grp_slot):
            oh[k, gcol * GRP : (gcol + 1) * GRP] = 1.0
        onehot_cores.append(oh)

        post.append((segs, pure_slots, tail_grp_slot))

    bounds = tuple(
        T_tail + max(
            (max((j + 1 for j, k in enumerate(psl) if k <= s), default=0))
            for psl in pure_slots_all
        )
        for s in range(S)
    )
    shi_pure = tuple(
        max(pure_slots_all[c][j] for c in range(N_CORES)) for j in range(P_pure)
    )
    return (
        (T_tail, P_pure, S, bounds, shi_pure),
        xT_cores, gmask_cores, umask2_cores, onehot_cores, post,
    )


def make_in_maps(inputs):
    x = np.asarray(inputs["x"], np.float32)
    seg_ids = np.asarray(inputs["seg_ids"])
    B = int(inputs["num_segments"])

    Wf, bf = [], []
    for i in (1, 2, 3, 4):
        W = np.asarray(inputs[f"W{i}"], np.float32)
        b = np.asarray(inputs[f"b{i}"], np.float32)
        ga = np.asarray(inputs[f"g{i}"], np.float32)
        be = np.asarray(inputs[f"be{i}"], np.float32)
        m = np.asarray(inputs[f"m{i}"], np.float32)
        v = np.asarray(inputs[f"v{i}"], np.float32)
        sc = ga / np.sqrt(v + EPS)
        Wf.append(np.ascontiguousarray(W * sc[None, :]))
        bf.append((b - m) * sc + be)
    W1f, W2f, W3f, W4f = Wf
    b1f, b2f, b3f, b4f = bf

    key, xT_cores, gmask_cores, umask2_cores, onehot_cores, post = _prepare(x, seg_ids, B)

    w1d = W1f.astype(np.float16)
    w2d = np.ascontiguousarray(W2f.reshape(128, 2, 128).astype(np.float16))
    w3ad = np.ascontiguousarray(
        W3f[:256].reshape(2, 128, 4, 128).transpose(1, 0, 2, 3).astype(np.float16)
    )
    w3bd = np.ascontiguousarray(
        W3f[256:].reshape(2, 128, 4, 128).transpose(1, 0, 2, 3).astype(np.float16)
    )
    w4d = np.ascontiguousarray(
        W4f.reshape(4, 128, 8, 128).transpose(1, 0, 2, 3).astype(np.float16)
    )
    b1d = np.ascontiguousarray(b1f.reshape(128, 1))
    b2d = np.ascontiguousarray(b2f.reshape(2, 128).T)
    b3d = np.ascontiguousarray(b3f.reshape(4, 128).T)

    in_maps = [
        {
            "xT": xT_cores[c],
            "gmask": gmask_cores[c],
            "umask2": umask2_cores[c],
            "onehot": onehot_cores[c],
            "w1": w1d,
            "w2": w2d,
            "w3a": w3ad,
            "w3b": w3bd,
            "w4": w4d,
            "b1": b1d,
            "b2": b2d,
            "b3": b3d,
        }
        for c in range(N_CORES)
    ]
    return key, in_maps, post, b4f


def postprocess(results, post, b4f, B, T_tail, P_pure):
    NTC = T_tail * NG
    out = np.zeros((B, 1024), np.float32)
    for c in range(N_CORES):
        mx4 = results[c]["mx4"]  # [128, 8, GW]
        segs, pure_slots, tail_grp_slot = post[c]
        for k, s in enumerate(segs):
            cols = [g for g, kk in enumerate(tail_grp_slot) if kk == k]
            cols += [NTC + j for j, kk in enumerate(pure_slots) if kk == k]
            raw = mx4[:, :, cols].max(axis=2)  # [128, 8]
            out[s] = np.maximum(raw.T.reshape(1024) + b4f, 0.0)
    return out


def get_program(key):
    if key not in _PROGRAM_CACHE:
        _PROGRAM_CACHE[key] = _build_program(*key)
    return _PROGRAM_CACHE[key]


def kernel(**inputs) -> np.ndarray:
    B = int(inputs["num_segments"])
    key, in_maps, post, b4f = make_in_maps(inputs)
    nc = get_program(key)
    last_err = None
    for _ in range(3):  # retry transient NRT device wedges
        try:
            res = run_bass_kernel_spmd(nc, in_maps, core_ids=list(range(N_CORES)))
            return postprocess(res.results, post, b4f, B, key[0], key[1])
        except Exception as e:  # noqa: BLE001
            last_err = e
    raise last_err
